# revision 1
# baseline (speedup 1.0000x reference)
"""GroupWiseTemporalAttention Trainium2 kernel.

Math: in the reference, SDPA runs with seq-len L=S=1 per channel-group, so
softmax over the single key is identically 1 and the attention output equals
v = (x+pe)_group @ v_w.T + v_b.  The whole module therefore folds into one
affine map:

    out = x_flat @ W_eff + b_eff
    W_eff = kron(I_192, v_w.T) @ proj_w.T            # [768, 768]
    b_eff = pe@W_eff + tile(v_b,192)@proj_w.T + proj_b

which we run as a data-parallel GEMM over 8 NeuronCores (6272 rows each).
The per-core kernel streams pre-transposed x^T tiles as the stationary
matmul operand so output lands in natural [tokens, channels] layout.
"""

import os

import numpy as np
import ml_dtypes

import concourse.bass as bass
import concourse.mybir as mybir
import concourse.tile as tile
from concourse import bacc
from concourse.bass_utils import run_bass_kernel_spmd

P = 128
C = 768
KC = C // P            # 6 contraction chunks
N_CORES = 8
B, H, W = 16, 56, 56
ROWS = B * H * W       # 50176
RPC = ROWS // N_CORES  # 6272 rows per core
TT = RPC // P          # 49 token tiles per core
TBLK = 4               # token tiles per input DMA block (512 tokens)
N_WARM = 6             # PE pre-warm matmuls issued during the DMA head

# Internal matmul dtype: bf16 halves input DMA and streams 1 col/cycle.
# fp32r keeps fp32 storage (full DMA) at 1 col/cycle for free-dim>=256.
VARIANT = os.environ.get("GWTA_VARIANT", "bf16")

LAST_STATS: dict = {}

_IN_DT = {
    "bf16": mybir.dt.bfloat16,
    "fp32r": mybir.dt.float32r,
    "fp32": mybir.dt.float32,
}


def _build_nc(variant: str) -> bass.Bass:
    in_dt = _IN_DT[variant]
    nc = bacc.Bacc(None, target_bir_lowering=False)
    xT = nc.declare_dram_parameter("xT", [C, RPC], in_dt, isOutput=False)
    w = nc.declare_dram_parameter("w", [C, C], in_dt, isOutput=False)
    b = nc.declare_dram_parameter("b", [P, C], mybir.dt.float32, isOutput=False)
    out = nc.declare_dram_parameter(
        "out", [RPC, C], mybir.dt.float32, isOutput=True
    )

    with tile.TileContext(nc) as tc:
        with (
            tc.tile_pool(name="const", bufs=1) as const,
            tc.tile_pool(name="xp", bufs=2) as xp,
            tc.tile_pool(name="op", bufs=4) as op,
            tc.tile_pool(name="pp", bufs=1, space="PSUM") as pp,
        ):
            # PE pre-warm: a few matmuls on zeroed SBUF keep the PE busy
            # during the DMA head so HAM un-throttles to 2.4GHz before the
            # real stream starts.  They borrow psum slot "pt3", which the
            # real stream touches last.
            g_rhs = const.tile([P, 512], in_dt)
            nc.vector.memset(g_rhs[:], 0.0)
            warm = pp.tile([P, C], mybir.dt.float32, tag="pt3")
            for _ in range(N_WARM):
                nc.tensor.matmul(
                    warm[:, 0:512], g_rhs[:, 0:P], g_rhs[:],
                    start=True, stop=True,
                )

            xTr = xT.rearrange("(kc p) t -> p kc t", p=P)

            # Weights resident in SBUF, one tile per contraction chunk, and
            # block 0's x chunks interleaved with them so the first real
            # matmul pair (w0, x0) waits on the minimum number of bytes.
            # Block-0 inputs go out on gpsimd's SWDGE ring: gpsimd's code
            # lands ~2us before the HWDGE sequencers start issuing, so the
            # critical first bytes arrive earlier.
            wts = []
            xts0 = []
            for kc in range(KC):
                wt = const.tile([P, C], in_dt, tag=f"w{kc}", name=f"w{kc}")
                nc.sync.dma_start(
                    out=wt[:], in_=w[kc * P : (kc + 1) * P, :]
                )
                wts.append(wt)
                xt = xp.tile([P, P], in_dt, tag=f"x{kc}", name=f"x{kc}0")
                nc.sync.dma_start(out=xt[:], in_=xTr[:, kc, 0:P])
                xts0.append(xt)
            # Bias replicated across partitions (host provides [128, C]);
            # needed late (first TT), so it rides the secondary ring.
            bt = const.tile([P, C], mybir.dt.float32)
            nc.scalar.dma_start(out=bt[:], in_=b[:])

            t0 = 0
            # 49 token tiles: a 1-tile block first (so the critical head
            # data is just W + 6 small x chunks), then 12 4-tile blocks.
            blocks = [1] + [TBLK] * ((TT - 1) // TBLK)
            assert sum(blocks) == TT
            for bi, nb in enumerate(blocks):
                if bi == 0:
                    def xslice(kc, s):
                        return xts0[kc][:, s * P : (s + 1) * P]
                else:
                    xt = xp.tile([P, KC, TBLK * P], in_dt, tag="xb", name="xb")
                    nc.scalar.dma_start(
                        out=xt[:, :, : nb * P],
                        in_=xTr[:, :, t0 * P : (t0 + nb) * P],
                    )

                    def xslice(kc, s, xt=xt):
                        return xt[:, kc, s * P : (s + 1) * P]

                pts = [
                    pp.tile(
                        [P, C], mybir.dt.float32, tag=f"pt{s}", name=f"pt{s}"
                    )
                    for s in range(nb)
                ]
                # s-outer: each psum group completes a quarter-block ahead
                # of the next, so bias-add TTs overlap the matmul stream.
                for s in range(nb):
                    for kc in range(KC):
                        lhsT = xslice(kc, s)
                        nc.tensor.matmul(
                            pts[s][:, 0:512], lhsT, wts[kc][:, 0:512],
                            start=(kc == 0), stop=(kc == KC - 1),
                        )
                        nc.tensor.matmul(
                            pts[s][:, 512:C], lhsT, wts[kc][:, 512:C],
                            start=(kc == 0), stop=(kc == KC - 1),
                        )
                for s in range(nb):
                    ot = op.tile([P, C], mybir.dt.float32, tag="ot")
                    # split at the PSUM bank boundary (one bank per DVE read)
                    nc.vector.tensor_add(
                        out=ot[:, 0:512], in0=pts[s][:, 0:512], in1=bt[:, 0:512]
                    )
                    nc.vector.tensor_add(
                        out=ot[:, 512:C], in0=pts[s][:, 512:C], in1=bt[:, 512:C]
                    )
                    nc.sync.dma_start(
                        out=out[(t0 + s) * P : (t0 + s + 1) * P, :], in_=ot[:]
                    )
                t0 += nb
    nc.compile()
    return nc


def _fold_weights(qkv_w, qkv_b, proj_w, proj_b, pe):
    v_w = qkv_w[2 * 4 : 3 * 4].astype(np.float64)   # [4, 4]
    v_b = qkv_b[2 * 4 : 3 * 4].astype(np.float64)   # [4]
    bd = np.kron(np.eye(C // 4), v_w.T)             # y_flat @ bd == groupwise v
    w_eff = bd @ proj_w.astype(np.float64).T        # [768, 768]
    b_eff = (
        np.tile(v_b, C // 4) @ proj_w.astype(np.float64).T
        + proj_b.astype(np.float64)
        + pe[:C].astype(np.float64) @ w_eff
    )
    return w_eff, b_eff


def _enable_tracing_shims():
    """Dev-only (GWTA_TRACE=1): restore the NTFF profile hook that this
    image's `antenv` is missing, and keep trace artifacts local instead of
    uploading.  Never active when the kernel is called normally."""
    import sys
    import types

    try:
        from antenv import axon_hooks  # noqa: F401
    except ImportError:
        import antenv
        from trn_agent_boot.trn_boot import _ntff_profile_via_ctypes

        mod = types.ModuleType("antenv.axon_hooks")
        mod._hook = _ntff_profile_via_ctypes("/opt/axon/libaxon_pjrt.so")
        mod.get_axon_ntff_profile_hook = lambda: mod._hook
        mod.set_axon_ntff_profile_hook = lambda h: setattr(mod, "_hook", h)
        sys.modules["antenv.axon_hooks"] = mod
        antenv.axon_hooks = mod

    import concourse.bass_utils as bu

    bu.upload_artifacts = lambda tmpdir: f"local:{tmpdir}"


def kernel(x, qkv_w, qkv_b, proj_w, proj_b, pe):
    x = np.asarray(x, np.float32)
    w_eff, b_eff = _fold_weights(
        np.asarray(qkv_w), np.asarray(qkv_b),
        np.asarray(proj_w), np.asarray(proj_b), np.asarray(pe),
    )

    variant = VARIANT
    if variant == "bf16":
        cast = lambda a: np.ascontiguousarray(a, dtype=ml_dtypes.bfloat16)
    else:
        cast = lambda a: np.ascontiguousarray(a, dtype=np.float32)

    w_dev = cast(w_eff)
    b_dev = np.broadcast_to(
        b_eff.astype(np.float32), (P, C)
    ).copy()

    x_flat = x.reshape(ROWS, C)
    in_maps = []
    for c in range(N_CORES):
        xt = cast(x_flat[c * RPC : (c + 1) * RPC].T)
        in_maps.append({"xT": xt, "w": w_dev, "b": b_dev})

    nc = _build_nc(variant)
    trace = bool(int(os.environ.get("GWTA_TRACE", "0")))
    kw = {}
    if trace:
        _enable_tracing_shims()
        kw["tmpdir"] = os.environ.get("GWTA_TRACE_DIR") or None
    r = run_bass_kernel_spmd(nc, in_maps, list(range(N_CORES)), trace=trace, **kw)

    LAST_STATS.clear()
    LAST_STATS.update(
        exec_time_ns=r.exec_time_ns,
        mean_exec_time_ns=r.mean_exec_time_ns,
        variant=variant,
    )

    out = np.empty((ROWS, C), np.float32)
    for c in range(N_CORES):
        out[c * RPC : (c + 1) * RPC] = r.results[c]["out"]
    return out.reshape(B, H, W, C)



# revision 5
# speedup vs baseline: 1.0259x; 1.0259x over previous
"""GroupWiseTemporalAttention Trainium2 kernel.

Math: in the reference, SDPA runs with seq-len L=S=1 per channel-group, so
softmax over the single key is identically 1 and the attention output equals
v = (x+pe)_group @ v_w.T + v_b.  The whole module therefore folds into one
affine map:

    out = x_flat @ W_eff + b_eff
    W_eff = kron(I_192, v_w.T) @ proj_w.T            # [768, 768]
    b_eff = pe@W_eff + tile(v_b,192)@proj_w.T + proj_b

which we run as a data-parallel GEMM over 8 NeuronCores (6272 rows each).
The per-core kernel streams pre-transposed x^T tiles as the stationary
matmul operand so output lands in natural [tokens, channels] layout.
"""

import os

import numpy as np
import ml_dtypes

import concourse.bass as bass
import concourse.mybir as mybir
import concourse.tile as tile
from concourse import bacc
from concourse.bass_utils import run_bass_kernel_spmd

P = 128
C = 768
KC = C // P            # 6 contraction chunks
N_CORES = 8
B, H, W = 16, 56, 56
ROWS = B * H * W       # 50176
RPC = ROWS // N_CORES  # 6272 rows per core
TT = RPC // P          # 49 token tiles per core
TBLK = 4               # token tiles per input DMA block (512 tokens)
N_WARM = 6             # PE pre-warm matmuls issued during the DMA head

# Internal matmul dtype: bf16 halves input DMA and streams 1 col/cycle.
# fp32r keeps fp32 storage (full DMA) at 1 col/cycle for free-dim>=256.
VARIANT = os.environ.get("GWTA_VARIANT", "bf16")

LAST_STATS: dict = {}

_IN_DT = {
    "bf16": mybir.dt.bfloat16,
    "fp32r": mybir.dt.float32r,
    "fp32": mybir.dt.float32,
}


def _build_nc(variant: str) -> bass.Bass:
    in_dt = _IN_DT[variant]
    nc = bacc.Bacc(None, target_bir_lowering=False)
    xT = nc.declare_dram_parameter("xT", [C, RPC], in_dt, isOutput=False)
    w = nc.declare_dram_parameter("w", [C, C], in_dt, isOutput=False)
    b = nc.declare_dram_parameter("b", [P, C], mybir.dt.float32, isOutput=False)
    # bf16 output halves the write stream (9.6MB/core vs 19.3MB); host upcasts.
    out = nc.declare_dram_parameter(
        "out", [RPC, C], mybir.dt.bfloat16, isOutput=True
    )

    with tile.TileContext(nc) as tc:
        with (
            tc.tile_pool(name="const", bufs=1) as const,
            tc.tile_pool(name="xp", bufs=2) as xp,
            tc.tile_pool(name="op", bufs=4) as op,
            tc.tile_pool(name="pp", bufs=1, space="PSUM") as pp,
        ):
            # PE pre-warm: a few matmuls on zeroed SBUF keep the PE busy
            # during the DMA head so HAM un-throttles to 2.4GHz before the
            # real stream starts.  They borrow psum slot "pt3", which the
            # real stream touches last.
            g_rhs = const.tile([P, 512], in_dt)
            nc.vector.memset(g_rhs[:], 0.0)
            warm = pp.tile([P, C], mybir.dt.float32, tag="pt3")
            for _ in range(N_WARM):
                nc.tensor.matmul(
                    warm[:, 0:512], g_rhs[:, 0:P], g_rhs[:],
                    start=True, stop=True,
                )

            xTr = xT.rearrange("(kc p) t -> p kc t", p=P)

            # Weights resident in SBUF, one tile per contraction chunk, and
            # block 0's x chunks interleaved with them so the first real
            # matmul pair (w0, x0) waits on the minimum number of bytes.
            # Block-0 inputs go out on gpsimd's SWDGE ring: gpsimd's code
            # lands ~2us before the HWDGE sequencers start issuing, so the
            # critical first bytes arrive earlier.
            wts = []
            xts0 = []
            for kc in range(KC):
                wt = const.tile([P, C], in_dt, tag=f"w{kc}", name=f"w{kc}")
                nc.sync.dma_start(
                    out=wt[:], in_=w[kc * P : (kc + 1) * P, :]
                )
                wts.append(wt)
                xt = xp.tile([P, P], in_dt, tag=f"x{kc}", name=f"x{kc}0")
                nc.sync.dma_start(out=xt[:], in_=xTr[:, kc, 0:P])
                xts0.append(xt)
            # Bias replicated across partitions (host provides [128, C]);
            # needed late (first TT).  It rides the sync ring AFTER the
            # W/x0 head so the scalar ring stays a pure input stream and
            # block-1's x doesn't queue behind 393KB of bias.
            bt = const.tile([P, C], mybir.dt.float32)
            nc.sync.dma_start(out=bt[:], in_=b[:])

            t0 = 0
            # 49 token tiles: a 1-tile block first (so the critical head
            # data is just W + 6 small x chunks), then 12 4-tile blocks.
            blocks = [1] + [TBLK] * ((TT - 1) // TBLK)
            assert sum(blocks) == TT
            for bi, nb in enumerate(blocks):
                if bi == 0:
                    def xslice(kc, s):
                        return xts0[kc][:, s * P : (s + 1) * P]
                else:
                    xt = xp.tile([P, KC, TBLK * P], in_dt, tag="xb", name="xb")
                    nc.scalar.dma_start(
                        out=xt[:, :, : nb * P],
                        in_=xTr[:, :, t0 * P : (t0 + nb) * P],
                    )

                    def xslice(kc, s, xt=xt):
                        return xt[:, kc, s * P : (s + 1) * P]

                pts = [
                    pp.tile(
                        [P, C], mybir.dt.float32, tag=f"pt{s}", name=f"pt{s}"
                    )
                    for s in range(nb)
                ]
                # s-outer: each psum group completes a quarter-block ahead
                # of the next, so bias-add TTs overlap the matmul stream.
                for s in range(nb):
                    for kc in range(KC):
                        lhsT = xslice(kc, s)
                        nc.tensor.matmul(
                            pts[s][:, 0:512], lhsT, wts[kc][:, 0:512],
                            start=(kc == 0), stop=(kc == KC - 1),
                        )
                        nc.tensor.matmul(
                            pts[s][:, 512:C], lhsT, wts[kc][:, 512:C],
                            start=(kc == 0), stop=(kc == KC - 1),
                        )
                for s in range(nb):
                    ot = op.tile([P, C], mybir.dt.bfloat16, tag="ot")
                    # split at the PSUM bank boundary (one bank per DVE read)
                    nc.vector.tensor_add(
                        out=ot[:, 0:512], in0=pts[s][:, 0:512], in1=bt[:, 0:512]
                    )
                    nc.vector.tensor_add(
                        out=ot[:, 512:C], in0=pts[s][:, 512:C], in1=bt[:, 512:C]
                    )
                    nc.sync.dma_start(
                        out=out[(t0 + s) * P : (t0 + s + 1) * P, :], in_=ot[:]
                    )
                t0 += nb
    nc.compile()
    return nc


def _fold_weights(qkv_w, qkv_b, proj_w, proj_b, pe):
    v_w = qkv_w[2 * 4 : 3 * 4].astype(np.float64)   # [4, 4]
    v_b = qkv_b[2 * 4 : 3 * 4].astype(np.float64)   # [4]
    bd = np.kron(np.eye(C // 4), v_w.T)             # y_flat @ bd == groupwise v
    w_eff = bd @ proj_w.astype(np.float64).T        # [768, 768]
    b_eff = (
        np.tile(v_b, C // 4) @ proj_w.astype(np.float64).T
        + proj_b.astype(np.float64)
        + pe[:C].astype(np.float64) @ w_eff
    )
    return w_eff, b_eff


def _enable_tracing_shims():
    """Dev-only (GWTA_TRACE=1): restore the NTFF profile hook that this
    image's `antenv` is missing, and keep trace artifacts local instead of
    uploading.  Never active when the kernel is called normally."""
    import sys
    import types

    try:
        from antenv import axon_hooks  # noqa: F401
    except ImportError:
        import antenv
        from trn_agent_boot.trn_boot import _ntff_profile_via_ctypes

        mod = types.ModuleType("antenv.axon_hooks")
        mod._hook = _ntff_profile_via_ctypes("/opt/axon/libaxon_pjrt.so")
        mod.get_axon_ntff_profile_hook = lambda: mod._hook
        mod.set_axon_ntff_profile_hook = lambda h: setattr(mod, "_hook", h)
        sys.modules["antenv.axon_hooks"] = mod
        antenv.axon_hooks = mod

    import concourse.bass_utils as bu

    bu.upload_artifacts = lambda tmpdir: f"local:{tmpdir}"


def kernel(x, qkv_w, qkv_b, proj_w, proj_b, pe):
    x = np.asarray(x, np.float32)
    w_eff, b_eff = _fold_weights(
        np.asarray(qkv_w), np.asarray(qkv_b),
        np.asarray(proj_w), np.asarray(proj_b), np.asarray(pe),
    )

    variant = VARIANT
    if variant == "bf16":
        cast = lambda a: np.ascontiguousarray(a, dtype=ml_dtypes.bfloat16)
    else:
        cast = lambda a: np.ascontiguousarray(a, dtype=np.float32)

    w_dev = cast(w_eff)
    b_dev = np.broadcast_to(
        b_eff.astype(np.float32), (P, C)
    ).copy()

    x_flat = x.reshape(ROWS, C)
    in_maps = []
    for c in range(N_CORES):
        xt = cast(x_flat[c * RPC : (c + 1) * RPC].T)
        in_maps.append({"xT": xt, "w": w_dev, "b": b_dev})

    nc = _build_nc(variant)
    trace = bool(int(os.environ.get("GWTA_TRACE", "0")))
    kw = {}
    if trace:
        _enable_tracing_shims()
        kw["tmpdir"] = os.environ.get("GWTA_TRACE_DIR") or None
    r = run_bass_kernel_spmd(nc, in_maps, list(range(N_CORES)), trace=trace, **kw)

    LAST_STATS.clear()
    LAST_STATS.update(
        exec_time_ns=r.exec_time_ns,
        mean_exec_time_ns=r.mean_exec_time_ns,
        variant=variant,
    )

    out = np.empty((ROWS, C), np.float32)
    for c in range(N_CORES):
        out[c * RPC : (c + 1) * RPC] = np.asarray(
            r.results[c]["out"]
        ).astype(np.float32)
    return out.reshape(B, H, W, C)

